# revision 1
# baseline (speedup 1.0000x reference)
"""Trainium2 Bass kernel for nn_BiDAF_Wemb.

Data-parallel over batch: 8 NeuronCores x 8 sequences each. Per core:
  attention (BiDAF) -> G.T in DRAM -> per layer: gi = x @ Wih.T (batched,
  fp32r) -> bidirectional GRU scan with h-stationary streaming matmuls,
  a 4-instruction DVE block-transpose into gates-on-partition layout,
  and fused elementwise gate math split across DVE/ACT/GPSIMD.

Self-contained: hardcodes all shapes; builds the Bass program on first call.
"""

import numpy as np

import bass_rust
import concourse.bass as bass
import concourse.mybir as mybir
import concourse.tile as tile_mod
from concourse.tile import TileContext
from concourse.bass_utils import run_bass_kernel_spmd

f32 = mybir.dt.float32
f32r = mybir.dt.float32r
AF = mybir.ActivationFunctionType
ALU = mybir.AluOpType

B, T, J, D = 64, 256, 64, 256
D2, H3 = 2 * D, 3 * D            # 512, 768
NCORES = 8
BL = B // NCORES                 # 8 sequences per core
NTOK = BL * T                    # 2048 tokens per core (t-major: col = t*BL + b)
NQTOK = BL * J                   # 512 query tokens (b-major: row = b*J + j)
IN_L = [8 * D, D2, 10 * D, D2]   # gi input widths per layer
NKL = [x // 128 for x in IN_L]   # K-chunks per layer: 16, 4, 20, 4


# ---------------------------------------------------------------------------
# toolchain patches: walrus in this container rejects >1 embedded sync-wait
# per instruction; split extras onto same-engine NoOp carriers.
# ---------------------------------------------------------------------------
def _patch_tile():
    if getattr(tile_mod.TileContext, "_bidaf_patched", False):
        return
    LIMIT = 1
    counter = [0]
    orig_lower = tile_mod.TileContext._lower_ordered_insts

    def split_list(insts):
        out = []
        for inst in insts:
            lim = 1
            si = inst.sync_info
            waits = list(si.on_wait) if si is not None else []
            if len(waits) > lim:
                rest = waits[lim:]
                for i in range(0, len(rest), lim):
                    counter[0] += 1
                    nop = mybir.InstNoOp(name=f"WS-{counter[0]}", engine=inst.engine)
                    nop.sync_info = bass_rust.SyncInfo(
                        on_wait=rest[i : i + lim], on_update=[]
                    )
                    out.append(nop)
                si.on_wait = waits[:lim]
                inst.sync_info = si
            out.append(inst)
        return out

    def patched_lower(self, ordered):
        for k in list(ordered.keys()):
            ordered[k] = split_list(ordered[k])
        return orig_lower(self, ordered)

    def patched_drain(self, tick_clock, wait_clock):
        nc = self.nc
        drain_inst = nc.sync.drain()
        wait_clock.add_sem_waits(
            drain_inst.ins, tile_mod.ScopedClock({None: tick_clock.global_clock})
        )
        si = drain_inst.ins.sync_info
        if si is not None and len(si.on_wait) > LIMIT:
            waits = list(si.on_wait)
            si.on_wait = waits[:LIMIT]
            drain_inst.ins.sync_info = si
            for i in range(LIMIT, len(waits), LIMIT):
                extra = nc.sync.drain()
                extra.ins.sync_info = bass_rust.SyncInfo(
                    on_wait=waits[i : i + LIMIT], on_update=[]
                )
        nc.all_engine_barrier()
        popped = nc._tile_sem_poison_stack.pop()
        assert popped is self._sem_poison
        nc.clear_and_free_semaphores(list(self.sems.allocated().values()))
        nc.all_engine_barrier()

    tile_mod.TileContext._lower_ordered_insts = patched_lower
    tile_mod.TileContext._drain_and_barrier = patched_drain
    tile_mod.TileContext._bidaf_patched = True


# ---------------------------------------------------------------------------
# program builder
# ---------------------------------------------------------------------------
def build_program(t_len=T, debug=False):
    _patch_tile()
    nt = BL * t_len          # tokens
    ntc = nt // 128          # 128-token chunks
    tcpb = t_len // 128      # t-chunks per sequence (2 at full size)

    nc = bass.Bass("TRN2", target_bir_lowering=False, debug=False)

    c_d = nc.dram_tensor("c", [nt, D2], f32, kind="ExternalInput")
    q_d = nc.dram_tensor("q", [NQTOK, D2], f32, kind="ExternalInput")
    eye_d = nc.dram_tensor("eye", [128, 128], f32, kind="ExternalInput")
    ws_d = nc.dram_tensor("wsplit", [3, D2], f32, kind="ExternalInput")
    wih_d = [
        nc.dram_tensor(f"wih{layer}", [IN_L[layer], 2 * H3], f32, kind="ExternalInput")
        for layer in range(4)
    ]
    whh_d = nc.dram_tensor("whhs", [4, 2, D, H3], f32, kind="ExternalInput")
    gb_d = nc.dram_tensor("gbias", [128, 4, 2, 6], f32, kind="ExternalInput")
    bhn_d = nc.dram_tensor("bhn", [128, 4, 2, 2], f32, kind="ExternalInput")
    zp_d = nc.dram_tensor("zpad", [128, 64], f32, kind="ExternalInput")
    wb_d = nc.dram_tensor("wbounce", [BL * 2, 128], f32)
    out_d = nc.dram_tensor("out", [BL, D2], f32, kind="ExternalOutput")
    gt_kind = "ExternalOutput" if debug else "Internal"
    gt_d = nc.dram_tensor("GT", [16, 128, nt], f32, kind=gt_kind)
    if debug:
        gi_d = nc.dram_tensor("dbg_gi", [128, t_len, 6, 16], f32, kind="ExternalOutput")
        y0_d = nc.dram_tensor("dbg_y0", [128, 4, nt], f32, kind="ExternalOutput")

    with TileContext(nc) as tc:
        with tc.tile_pool(name="const", bufs=1) as pc:
            eyesb = pc.tile([128, 128], f32, tag="eye")
            wsb = pc.tile([128, 3, 4], f32, tag="wsb")        # [p, (wc,wq,wm), dchunk]
            gbsb = pc.tile([128, 4, 2, 6], f32, tag="gbsb")
            bhnsb = pc.tile([128, 4, 2, 2], f32, tag="bhnsb")
            zeros = pc.tile([128, 2, BL], f32r, tag="zeros")
            onesr = pc.tile([1, 128], f32, tag="onesr")
            nc.sync.dma_start(out=eyesb[:], in_=eye_d[:])
            nc.sync.dma_start(
                out=wsb[:], in_=ws_d[:].rearrange("i (dc p) -> p i dc", p=128)
            )
            nc.sync.dma_start(out=gbsb[:], in_=gb_d[:])
            nc.sync.dma_start(out=bhnsb[:], in_=bhn_d[:])
            nc.sync.dma_start(
                out=zeros[:],
                in_=zp_d[:, 0 : 2 * BL].rearrange("p (k b) -> p k b", k=2).bitcast(f32r),
            )
            nc.vector.memset(onesr[:], 1.0)

            # ===============================================================
            # PHASE A: attention -> G.T (DRAM)
            # ===============================================================
            with (
                tc.tile_pool(name="attn", bufs=1) as pa,
                tc.tile_pool(name="attn2", bufs=2) as pa2,
                tc.tile_pool(name="psA", bufs=4, space="PSUM") as psA,
            ):
                cnat = pa.tile([128, ntc, D2], f32, tag="cnat")
                qnat = pa.tile([64, BL, D2], f32, tag="qnat")
                cT = pa.tile([128, 4, nt], f32, tag="cT")
                c2qT = pa.tile([128, 4, nt], f32, tag="c2qT")
                qT = pa.tile([128, 4, NQTOK], f32, tag="qT")
                qmT = pa.tile([128, 4, NQTOK], f32, tag="qmT")
                for ch in range(ntc):
                    nc.sync.dma_start(
                        out=cnat[:, ch, :], in_=c_d[128 * ch : 128 * (ch + 1), :]
                    )
                nc.sync.dma_start(
                    out=qnat[:], in_=q_d[:].rearrange("(b j) d -> j b d", j=J)
                )

                qnat2 = pa.tile([128, 4, D2], f32, tag="qnat2")
                for ch in range(4):
                    nc.sync.dma_start(
                        out=qnat2[:, ch, :], in_=q_d[128 * ch : 128 * (ch + 1), :]
                    )
                # transposes: cT[p=d%128, dc, tok], qT[p, dc, qtok]
                for ch in range(ntc):
                    for dc in range(4):
                        pt = psA.tile([128, 128], f32, tag="aps")
                        nc.tensor.transpose(
                            pt[:], cnat[:, ch, 128 * dc : 128 * (dc + 1)], eyesb[:]
                        )
                        nc.vector.tensor_copy(
                            cT[:, dc, 128 * ch : 128 * (ch + 1)], pt[:]
                        )
                for ch in range(4):
                    for dc in range(4):
                        pt = psA.tile([128, 128], f32, tag="aps")
                        nc.tensor.transpose(
                            pt[:], qnat2[:, ch, 128 * dc : 128 * (dc + 1)], eyesb[:]
                        )
                        nc.vector.tensor_copy(
                            qT[:, dc, 128 * ch : 128 * (ch + 1)], pt[:]
                        )

                # qmT = qT * wm + wc  (folds the c@wc rank-1 term into S)
                for dc in range(4):
                    nc.vector.tensor_scalar(
                        out=qmT[:, dc, :],
                        in0=qT[:, dc, :],
                        scalar1=wsb[:, 2, dc : dc + 1],
                        scalar2=wsb[:, 0, dc : dc + 1],
                        op0=ALU.mult,
                        op1=ALU.add,
                    )

                # v row: q @ wq  -> [1, NQTOK]
                vps = psA.tile([1, NQTOK], f32, tag="aps")
                for dc in range(4):
                    nc.tensor.matmul(
                        vps[:],
                        wsb[:, 1, dc : dc + 1],
                        qT[:, dc, :],
                        start=(dc == 0),
                        stop=(dc == 3),
                    )
                vrow = pa.tile([1, NQTOK], f32, tag="vrow")
                nc.vector.tensor_copy(vrow[:], vps[:])

                # S per (b, tchunk): S = cT.T @ qmT + v  -> softmax over J
                S2 = pa.tile([128, ntc, J], f32, tag="S2")
                nm = pa.tile([128, ntc], f32, tag="nm")
                sums = pa.tile([128, ntc], f32, tag="sums")
                rs = pa.tile([128, ntc], f32, tag="rs")
                AT = pa.tile([64, ntc, 128], f32, tag="AT")
                for b in range(BL):
                    for tch in range(tcpb):
                        col = b * tcpb + tch
                        sps = psA.tile([128, J], f32, tag="aps")
                        for dc in range(4):
                            st0 = b + 8 * 128 * tch
                            lhsT = cT[:, dc, st0 : st0 + 8 * 127 + 1 : 8]
                            nc.tensor.matmul(
                                sps[:],
                                lhsT,
                                qmT[:, dc, 64 * b : 64 * (b + 1)],
                                start=(dc == 0),
                                stop=False,
                            )
                        nc.tensor.matmul(
                            sps[:],
                            onesr[:],
                            vrow[0:1, 64 * b : 64 * (b + 1)],
                            start=False,
                            stop=True,
                        )
                        nc.vector.tensor_reduce(
                            nm[:, col : col + 1],
                            sps[:],
                            axis=mybir.AxisListType.X,
                            op=ALU.max,
                            negate=True,
                        )
                        nc.scalar.activation(
                            S2[:, col, :],
                            sps[:],
                            AF.Exp,
                            bias=nm[:, col : col + 1],
                            accum_out=sums[:, col : col + 1],
                        )
                nc.vector.reciprocal(rs[:, :], sums[:, :])
                for b in range(BL):
                    for tch in range(tcpb):
                        col = b * tcpb + tch
                        asc = pa2.tile([128, J], f32, tag="asc")
                        nc.vector.scalar_tensor_tensor(
                            asc[:],
                            S2[:, col, :],
                            rs[:, col : col + 1],
                            S2[:, col, :],
                            ALU.mult,
                            ALU.bypass,
                        )
                        atp = psA.tile([64, 128], f32, tag="aps")
                        nc.tensor.transpose(atp[:], asc[:], eyesb[:])
                        nc.vector.tensor_copy(AT[0:64, col, :], atp[:])

                # c2qT[d, tok] = q.T @ A.T  (per b)
                for b in range(BL):
                    for m in range(4):
                        cps = psA.tile([128, 128 * tcpb], f32, tag="aps")
                        lhsT = qnat[:, b, 128 * m : 128 * (m + 1)]
                        nc.tensor.matmul(
                            cps[:],
                            lhsT,
                            AT[0:64, b * tcpb : (b + 1) * tcpb, :],
                            start=True,
                            stop=True,
                        )
                        nc.vector.tensor_copy(
                            c2qT[:, m, b : nt : 8].rearrange(
                                "p (a c) -> p a c", a=tcpb
                            ),
                            cps[:].rearrange("p (a c) -> p a c", a=tcpb),
                        )

                # b_att = softmax_T(max_J S); mt holds max_J S = -nm
                mt = pa.tile([128, ntc], f32, tag="mt")
                nc.scalar.mul(mt[:, :], nm[:, :], -1.0)
                mtp = psA.tile([ntc, 128], f32, tag="aps")
                nc.tensor.transpose(mtp[:], mt[:, :], eyesb[:])
                mtT = pa.tile([ntc, 128], f32, tag="mtT")
                nc.vector.tensor_copy(mtT[:], mtp[:])

                s16 = pa.tile([ntc, 1], f32, tag="s16")
                nc.scalar.activation(
                    mtT[:], mtT[:], AF.Exp, bias=0.0, accum_out=s16[:]
                )
                wsc = pa.tile([ntc, 1], f32, tag="wsc")
                if tcpb > 1:
                    # pair-sum (b, tch) rows in the free dim of a 1-partition row
                    srp = psA.tile([1, ntc], f32, tag="aps", name="srp")
                    nc.tensor.transpose(srp[:], s16[:], eyesb[0:ntc, 0:ntc])
                    srow = pa.tile([1, ntc], f32, tag="srow")
                    nc.vector.tensor_copy(srow[:], srp[:])
                    zrow = pa.tile([1, BL], f32, tag="zrow")
                    nc.vector.tensor_tensor(
                        zrow[:], srow[0:1, 0:ntc:2], srow[0:1, 1:ntc:2], ALU.add
                    )
                    rrow = pa.tile([1, BL], f32, tag="rrow")
                    nc.vector.reciprocal(rrow[:], zrow[:])
                    r2row = pa.tile([1, ntc], f32, tag="r2row")
                    nc.vector.tensor_copy(r2row[0:1, 0:ntc:2], rrow[:])
                    nc.vector.tensor_copy(r2row[0:1, 1:ntc:2], rrow[:])
                    wsp = psA.tile([ntc, 1], f32, tag="aps", name="wsp")
                    nc.tensor.transpose(wsp[:], r2row[:], eyesb[0:1, 0:1])
                    nc.vector.tensor_copy(wsc[:], wsp[:])
                else:
                    nc.vector.reciprocal(wsc[:], s16[:])
                w16 = pa.tile([ntc, 128], f32, tag="w16")
                nc.vector.scalar_tensor_tensor(
                    w16[:], mtT[:], wsc[:], mtT[:], ALU.mult, ALU.bypass
                )
                wtp = psA.tile([128, ntc], f32, tag="aps")
                nc.tensor.transpose(wtp[:], w16[:], eyesb[0:ntc, 0:ntc])
                wT = pa.tile([128, ntc], f32, tag="wT")
                nc.vector.tensor_copy(wT[:], wtp[:])

                # q2c[b] = sum_t w[b,t] c[b,t,:], with t-major tokens:
                # masked contraction over all tokens. wmask[p, ch, b] = w(token)
                # iff token%8==b. Token tok=128ch+p, t=tok//8; w16 flat index
                # = 256*b + t (tcpb=2) i.e. tcpb*128*b + t.
                nc.sync.dma_start(out=wb_d[0 : ntc, :], in_=w16[:, :])
                wmask = pa.tile([128, ntc, BL], f32, tag="wmask")
                nc.vector.memset(wmask[:], 0.0)
                wbflat = wb_d[:].rearrange("a c -> (a c)")
                for b in range(BL):
                    nc.sync.dma_start(
                        out=wmask[b : 121 + b : 8, :, b],
                        in_=wbflat[
                            tcpb * 128 * b : tcpb * 128 * (b + 1)
                        ].rearrange("(ch j) -> j ch", j=16),
                    )
                q2ps = psA.tile([BL, D2], f32, tag="q2ps")
                for ch in range(ntc):
                    nc.tensor.matmul(
                        q2ps[:],
                        wmask[:, ch, :],
                        cnat[:, ch, :],
                        start=(ch == 0),
                        stop=(ch == ntc - 1),
                    )
                q2c8 = pa.tile([BL, D2], f32, tag="q2c8")
                nc.vector.tensor_copy(q2c8[:], q2ps[:])
                q2cT = pa.tile([128, 4, BL], f32, tag="q2cT")
                for dc in range(4):
                    qtp = psA.tile([128, BL], f32, tag="aps")
                    nc.tensor.transpose(
                        qtp[:], q2c8[:, 128 * dc : 128 * (dc + 1)], eyesb[0:BL, 0:BL]
                    )
                    nc.vector.tensor_copy(q2cT[:, dc, :], qtp[:])

                # emit G.T = [c; c2q; c*c2q; c*q2c].T to DRAM
                for dc in range(4):
                    nc.sync.dma_start(out=gt_d[dc, :, :], in_=cT[:, dc, :])
                    nc.sync.dma_start(out=gt_d[4 + dc, :, :], in_=c2qT[:, dc, :])
                for dc in range(4):
                    nc.vector.tensor_tensor(
                        c2qT[:, dc, :], cT[:, dc, :], c2qT[:, dc, :], ALU.mult
                    )
                    nc.sync.dma_start(out=gt_d[8 + dc, :, :], in_=c2qT[:, dc, :])
                for dc in range(4):
                    cview = cT[:, dc, :].rearrange("p (t b) -> p t b", b=BL)
                    bview = q2cT[:, dc, :].rearrange("p (o b) -> p o b", o=1).broadcast_to(
                        [128, t_len, BL]
                    )
                    nc.vector.tensor_tensor(cview, cview, bview, ALU.mult)
                    nc.sync.dma_start(out=gt_d[12 + dc, :, :], in_=cT[:, dc, :])

            # ===============================================================
            # PHASE B: 4 biGRU layers (gi batched matmul + scan)
            # ===============================================================
            with (
                tc.tile_pool(name="main", bufs=1) as pm,
                tc.tile_pool(name="wst", bufs=8) as pw,
                tc.tile_pool(name="gst", bufs=3) as pg,
                tc.tile_pool(name="scr", bufs=2) as pscr,
                tc.tile_pool(name="gips", bufs=1, space="PSUM") as pgi,
                tc.tile_pool(name="scps", bufs=1, space="PSUM") as pscan,
            ):
                gi = pm.tile([128, t_len, 6, 16], f32, tag="gi")
                yT = pm.tile([128, 4, nt], f32r, tag="yT")
                whhs = pm.tile([128, 2, 2, H3], f32r, tag="whhs")

                SL = min(512, nt)
                gacc = [
                    pgi.tile([128, SL], f32, tag=f"gacc{i}", name=f"gacc{i}")
                    for i in range(6)
                ]
                pA = pscan.tile([32, 384], f32, tag="pA")
                pB = pscan.tile([32, 384], f32, tag="pB")
                hpad = pm.tile([128, 4, 16], f32r, tag="hpad")
                bhnt = pm.tile([128, 2, 16], f32, tag="bhnt")

                nsl = nt // SL

                for layer in range(4):
                    # --- load this layer's stream weights
                    nc.sync.dma_start(
                        out=whhs[:],
                        in_=whh_d[layer].rearrange(
                            "d (kc p) n -> p d kc n", p=128
                        ).bitcast(f32r),
                    )

                    # --- gi build: gi[:, t, ch, d*8+b] = (x @ Wih.T + bias)
                    nk = NKL[layer]
                    for half in range(2):
                        d = half  # m-tiles 0..5 are fwd, 6..11 bwd
                        for s in range(nsl):
                            for ki in range(nk):
                                if layer == 0 or (layer == 2 and ki < 16):
                                    rt = pg.tile([128, SL], f32r, tag="gs")
                                    nc.sync.dma_start(
                                        out=rt[:],
                                        in_=gt_d[ki, :, SL * s : SL * (s + 1)].bitcast(
                                            f32r
                                        ),
                                    )
                                    rhs = rt[:]
                                else:
                                    kc = ki - 16 if layer == 2 else ki
                                    rhs = yT[:, kc, SL * s : SL * (s + 1)]
                                for mm in range(6):
                                    m = 6 * half + mm
                                    wt = pw.tile([128, 128], f32r, tag="wt")
                                    nc.sync.dma_start(
                                        out=wt[:],
                                        in_=wih_d[layer][
                                            128 * ki : 128 * (ki + 1),
                                            128 * m : 128 * (m + 1),
                                        ].bitcast(f32r),
                                    )
                                    nc.tensor.matmul(
                                        gacc[mm][:],
                                        wt[:],
                                        rhs,
                                        start=(ki == 0),
                                        stop=(ki == nk - 1),
                                    )
                            for mm in range(6):
                                ch = mm
                                slt = SL // 8
                                gslice = gi[
                                    :,
                                    slt * s : slt * (s + 1),
                                    ch,
                                    8 * d : 8 * (d + 1),
                                ]
                                nc.scalar.activation(
                                    gslice,
                                    gacc[mm][:].rearrange("p (a b) -> p a b", b=8),
                                    AF.Identity,
                                    bias=gbsb[:, layer, d, ch : ch + 1],
                                )

                    if debug and layer == 0:
                        nc.sync.dma_start(out=gi_d[:], in_=gi[:])
                    # --- scan (fused fwd+bwd chain; h lives in hpad diag)
                    last = layer == 3
                    nc.sync.dma_start(
                        out=hpad[:],
                        in_=zp_d[:].rearrange("p (k b) -> p k b", k=4).bitcast(f32r),
                    )
                    bhp = bhnsb[:, layer, 0, :].rearrange("p (c o) -> p c o", o=1)
                    nc.vector.tensor_copy(
                        bhnt[:, :, 0:8], bhp.broadcast_to([128, 2, 8])
                    )
                    bhp = bhnsb[:, layer, 1, :].rearrange("p (c o) -> p c o", o=1)
                    nc.vector.tensor_copy(
                        bhnt[:, :, 8:16], bhp.broadcast_to([128, 2, 8])
                    )
                    hdiag = (hpad[:, 0:2, 0:8], hpad[:, 2:4, 8:16])
                    for step in range(t_len):
                        tf, tb = step, t_len - 1 - step
                        for hn, ptile in ((0, pA), (1, pB)):
                            for kc in range(4):
                                nc.tensor.matmul(
                                    ptile[0:16, :],
                                    hpad[:, kc, :],
                                    whhs[:, kc // 2, kc % 2, 384 * hn : 384 * (hn + 1)],
                                    start=(kc == 0),
                                    stop=(kc == 3),
                                )
                        gT = pscr.tile([128, 6, 32], f32, tag="gT")
                        for X, ptile in ((0, pA), (1, pB)):
                            pv = ptile[:].rearrange("p (c j) -> p c j", j=32)
                            for e in range(2):
                                off = 32 * (2 * X + e)
                                nc.vector.transpose(
                                    gT[off : off + 32, :, :], pv[:, e::2, :]
                                )
                        ntl = pscr.tile([128, 2, 16], f32, tag="ntl")
                        dtl = pscr.tile([128, 2, 16], f32, tag="dtl")
                        for dd, th in ((0, tf), (1, tb)):
                            cs = 8 * dd
                            nc.vector.tensor_tensor(
                                gT[:, 0:4, cs : cs + 8],
                                gT[:, 0:4, cs : cs + 8],
                                gi[:, th, 0:4, cs : cs + 8],
                                ALU.add,
                            )
                        nc.scalar.activation(
                            gT[:, 0:4, 0:16], gT[:, 0:4, 0:16], AF.Sigmoid
                        )
                        # n = tanh(i_n + r*(h_n + b_hn))
                        nc.vector.tensor_tensor(
                            ntl[:], gT[:, 4:6, 0:16], bhnt[:], ALU.add
                        )
                        nc.vector.tensor_tensor(
                            ntl[:], ntl[:], gT[:, 0:2, 0:16], ALU.mult
                        )
                        for dd, th in ((0, tf), (1, tb)):
                            cs = 8 * dd
                            nc.vector.tensor_tensor(
                                ntl[:, :, cs : cs + 8],
                                ntl[:, :, cs : cs + 8],
                                gi[:, th, 4:6, cs : cs + 8],
                                ALU.add,
                            )
                        nc.scalar.activation(ntl[:], ntl[:], AF.Tanh)
                        # h = n + z*(h_prev - n); h_prev/h live in hpad diag
                        for dd, th in ((0, tf), (1, tb)):
                            cs = 8 * dd
                            nc.vector.tensor_tensor(
                                dtl[:, :, cs : cs + 8],
                                hdiag[dd],
                                ntl[:, :, cs : cs + 8],
                                ALU.subtract,
                            )
                            nc.vector.tensor_tensor(
                                dtl[:, :, cs : cs + 8],
                                gT[:, 2:4, cs : cs + 8],
                                dtl[:, :, cs : cs + 8],
                                ALU.mult,
                            )
                            nc.vector.tensor_tensor(
                                hdiag[dd],
                                ntl[:, :, cs : cs + 8],
                                dtl[:, :, cs : cs + 8],
                                ALU.add,
                            )
                            if not last:
                                nc.gpsimd.tensor_copy(
                                    yT[:, 2 * dd : 2 * dd + 2, BL * th : BL * (th + 1)],
                                    hdiag[dd],
                                )

                    if debug and layer == 0:
                        nc.sync.dma_start(out=y0_d[:], in_=yT[:].bitcast(f32))
                # output: [hb, hf] per sequence (finals live in hpad diag)
                for hh, dd in ((0, 1), (1, 0)):
                    for chh in range(2):
                        c0 = 256 * hh + 128 * chh
                        ov = out_d[:, c0 : c0 + 128].rearrange("b p -> p b")
                        nc.sync.dma_start(
                            out=ov,
                            in_=hpad[:, 2 * dd + chh, 8 * dd : 8 * dd + 8].bitcast(f32),
                        )

    return nc


# ---------------------------------------------------------------------------
# host-side weight prep
# ---------------------------------------------------------------------------
def _perm768():
    m = np.arange(H3)
    return (
        (m % 384) // 64 * 128 + 64 * (m // 384) + 32 * ((m % 64) // 32) + m % 32
    )


def _prep_weights(inputs):
    perm = _perm768()
    names = ["mod0", "mod1", "rep0", "rep1"]
    wih, whhs = [], np.empty((4, 2, D, H3), np.float32)
    gb = np.empty((128, 4, 2, 6), np.float32)
    bhn = np.empty((128, 4, 2, 2), np.float32)
    for layer, nm in enumerate(names):
        Wih = np.asarray(inputs[f"{nm}_Wih"], np.float32)   # [2, 768, in]
        Whh = np.asarray(inputs[f"{nm}_Whh"], np.float32)   # [2, 768, 256]
        bb = np.asarray(inputs[f"{nm}_b"], np.float32)      # [2, 2, 768]
        wih.append(np.ascontiguousarray(np.concatenate([Wih[0].T, Wih[1].T], axis=1)))
        for d in range(2):
            whhs[layer, d] = np.ascontiguousarray(Whh[d][perm, :].T)
            vec = bb[d, 0] + np.concatenate([bb[d, 1][:D2], np.zeros(D, np.float32)])
            gb[:, layer, d, :] = vec.reshape(6, 128).T
            bhn[:, layer, d, :] = bb[d, 1][D2:].reshape(2, 128).T
    return wih, whhs, gb, bhn


_PROG = None


def kernel(**inputs):
    global _PROG
    if _PROG is None:
        _PROG = build_program()
    nc = _PROG

    wih, whhs, gb, bhn = _prep_weights(inputs)
    ws = np.asarray(inputs["Ws"], np.float32).reshape(3, D2)
    eye = np.eye(128, dtype=np.float32)
    c_all = np.asarray(inputs["embd_context"], np.float32)
    q_all = np.asarray(inputs["embd_query"], np.float32)

    shared = {
        "eye": eye,
        "zpad": np.zeros((128, 64), np.float32),
        "wsplit": np.ascontiguousarray(ws),
        "whhs": whhs,
        "gbias": gb,
        "bhn": bhn,
    }
    for layer in range(4):
        shared[f"wih{layer}"] = wih[layer]

    in_maps = []
    for i in range(NCORES):
        ci = c_all[BL * i : BL * (i + 1)]           # [8, 256, 512]
        c_tm = np.ascontiguousarray(
            ci.transpose(1, 0, 2).reshape(T * BL, D2)
        )
        qi = np.ascontiguousarray(
            q_all[BL * i : BL * (i + 1)].reshape(NQTOK, D2)
        )
        m = dict(shared)
        m["c"] = c_tm
        m["q"] = qi
        in_maps.append(m)

    res = run_bass_kernel_spmd(nc, in_maps, core_ids=list(range(NCORES)))
    out = np.concatenate([res.results[i]["out"] for i in range(NCORES)], axis=0)
    return np.ascontiguousarray(out.astype(np.float32))



# revision 15
# speedup vs baseline: 1.4698x; 1.4698x over previous
"""Trainium2 Bass kernel for nn_BiDAF_Wemb.

Data-parallel over batch: 8 NeuronCores x 8 sequences each. Per core:
  attention (BiDAF) -> G.T in DRAM -> per layer: gi = x @ Wih.T (batched,
  fp32r) -> bidirectional GRU scan with h-stationary streaming matmuls,
  a 4-instruction DVE block-transpose into gates-on-partition layout,
  and fused elementwise gate math split across DVE/ACT/GPSIMD.

Self-contained: hardcodes all shapes; builds the Bass program on first call.
"""

import numpy as np

import bass_rust
import concourse.bass as bass
import concourse.mybir as mybir
import concourse.tile as tile_mod
from concourse.tile import TileContext
from concourse.tile_rust import add_dep_helper
from concourse.bass_utils import run_bass_kernel_spmd

f32 = mybir.dt.float32
f32r = mybir.dt.float32r
bf16 = mybir.dt.bfloat16
AF = mybir.ActivationFunctionType
ALU = mybir.AluOpType

B, T, J, D = 64, 256, 64, 256
D2, H3 = 2 * D, 3 * D            # 512, 768
NCORES = 8
BL = B // NCORES                 # 8 sequences per core
NTOK = BL * T                    # 2048 tokens per core (t-major: col = t*BL + b)
NQTOK = BL * J                   # 512 query tokens (b-major: row = b*J + j)
IN_L = [8 * D, D2, 10 * D, D2]   # gi input widths per layer
NKL = [x // 128 for x in IN_L]   # K-chunks per layer: 16, 4, 20, 4


# ---------------------------------------------------------------------------
# toolchain patches: walrus in this container rejects >1 embedded sync-wait
# per instruction; split extras onto same-engine NoOp carriers.
# ---------------------------------------------------------------------------
def _patch_tile():
    if getattr(tile_mod.TileContext, "_bidaf_patched", False):
        return
    LIMIT = 1
    counter = [0]
    orig_lower = tile_mod.TileContext._lower_ordered_insts

    def split_list(nc, insts):
        # Walk in scheduled order tracking per-semaphore emitted updates.
        # For multi-wait instructions keep the binding wait (min slack)
        # embedded; other waits ride on ENGINE_NOP carriers, which wait in
        # the engine wait-queue rather than blocking the sequencer.
        eng_map = {
            mybir.EngineType.PE: nc.tensor,
            mybir.EngineType.DVE: nc.vector,
            mybir.EngineType.Activation: nc.scalar,
            mybir.EngineType.Pool: nc.gpsimd,
            mybir.EngineType.SP: nc.sync,
        }
        nop_op = nc.isa.Opcode.NEURON_ISA_TPB_OPCODE_ENGINE_NOP
        counts = {}
        out = []
        for inst in insts:
            lim = 1
            si = inst.sync_info
            waits = list(si.on_wait) if si is not None else []
            if len(waits) > lim:
                def slack(w):
                    v = w.wait_value if isinstance(w.wait_value, int) else 0
                    return counts.get((w.sync_type, w.id), 0) - v
                waits.sort(key=slack)
                rest = waits[lim:]
                for i in range(0, len(rest), lim):
                    counter[0] += 1
                    nop = mybir.InstNoOp(name=f"WS-{counter[0]}", engine=inst.engine)
                    nop.sync_info = bass_rust.SyncInfo(
                        on_wait=rest[i : i + lim], on_update=[]
                    )
                    out.append(nop)
                si.on_wait = waits[:lim]
                inst.sync_info = si
            if si is not None:
                for u in si.on_update:
                    v = u.update_value if isinstance(u.update_value, int) else 0
                    k = (u.sync_type, u.id)
                    counts[k] = counts.get(k, 0) + v
            out.append(inst)
        return out

    def patched_lower(self, ordered):
        for k in list(ordered.keys()):
            ordered[k] = split_list(self.nc, ordered[k])
        return orig_lower(self, ordered)

    def patched_drain(self, tick_clock, wait_clock):
        nc = self.nc
        drain_inst = nc.sync.drain()
        wait_clock.add_sem_waits(
            drain_inst.ins, tile_mod.ScopedClock({None: tick_clock.global_clock})
        )
        si = drain_inst.ins.sync_info
        if si is not None and len(si.on_wait) > LIMIT:
            waits = list(si.on_wait)
            si.on_wait = waits[:LIMIT]
            drain_inst.ins.sync_info = si
            for i in range(LIMIT, len(waits), LIMIT):
                extra = nc.sync.drain()
                extra.ins.sync_info = bass_rust.SyncInfo(
                    on_wait=waits[i : i + LIMIT], on_update=[]
                )
        nc.all_engine_barrier()
        popped = nc._tile_sem_poison_stack.pop()
        assert popped is self._sem_poison
        nc.clear_and_free_semaphores(list(self.sems.allocated().values()))
        nc.all_engine_barrier()

    tile_mod.TileContext._lower_ordered_insts = patched_lower
    tile_mod.TileContext._drain_and_barrier = patched_drain
    tile_mod.TileContext._bidaf_patched = True


# ---------------------------------------------------------------------------
# program builder
# ---------------------------------------------------------------------------
def build_program(t_len=T, debug=False):
    _patch_tile()
    nt = BL * t_len          # tokens
    ntc = nt // 128          # 128-token chunks
    tcpb = t_len // 128      # t-chunks per sequence (2 at full size)

    nc = bass.Bass("TRN2", target_bir_lowering=False, debug=False)

    c_d = nc.dram_tensor("c", [nt, D2], f32, kind="ExternalInput")
    q_d = nc.dram_tensor("q", [NQTOK, D2], f32, kind="ExternalInput")
    eye_d = nc.dram_tensor("eye", [128, 128], f32, kind="ExternalInput")
    ws_d = nc.dram_tensor("wsplit", [3, D2], f32, kind="ExternalInput")
    wih_d = [
        nc.dram_tensor(f"wih{layer}", [IN_L[layer], 2 * H3], f32, kind="ExternalInput")
        for layer in range(4)
    ]
    whh_d = nc.dram_tensor("whhs", [128, 4, 2, 2, 6, 128], bf16, kind="ExternalInput")
    gb_d = nc.dram_tensor("gbias", [128, 4, 2, 6], f32, kind="ExternalInput")
    bhnt_d = nc.dram_tensor("bhnt", [128, 4, 2, 16], bf16, kind="ExternalInput")
    eyeb_d = nc.dram_tensor("eyeb", [128, 128], bf16, kind="ExternalInput")
    wb_d = nc.dram_tensor("wbounce", [BL * 2, 128], f32)
    out_d = nc.dram_tensor("out", [BL, D2], f32, kind="ExternalOutput")
    gt_d = nc.dram_tensor("GT", [16, 128, nt], f32, kind="Internal")

    with TileContext(nc) as tc:
        with tc.tile_pool(name="const", bufs=1) as pc:
            eyesb = pc.tile([128, 128], f32, tag="eye")
            eyeb = pc.tile([128, 128], bf16, tag="eyeb")
            wsb = pc.tile([128, 3, 4], f32, tag="wsb")        # [p, (wc,wq,wm), dchunk]
            gbsb = pc.tile([128, 4, 2, 6], f32, tag="gbsb")
            bhnt = pc.tile([128, 4, 2, 16], bf16, tag="bhnt")
            whhs = pc.tile([128, 4, 2, 2, 6, 128], bf16, tag="whhs")
            onesr = pc.tile([1, 128], f32, tag="onesr")
            nc.sync.dma_start(out=eyesb[:], in_=eye_d[:])
            nc.sync.dma_start(out=eyeb[:], in_=eyeb_d[:])
            nc.sync.dma_start(
                out=wsb[:], in_=ws_d[:].rearrange("i (dc p) -> p i dc", p=128)
            )
            nc.sync.dma_start(out=gbsb[:], in_=gb_d[:])
            nc.sync.dma_start(out=bhnt[:], in_=bhnt_d[:])
            nc.sync.dma_start(out=whhs[:], in_=whh_d[:])
            nc.vector.memset(onesr[:], 1.0)

            # ===============================================================
            # PHASE A: attention -> G.T (DRAM)
            # ===============================================================
            with (
                tc.tile_pool(name="attn", bufs=1) as pa,
                tc.tile_pool(name="attn2", bufs=2) as pa2,
                tc.tile_pool(name="psA", bufs=4, space="PSUM") as psA,
            ):
                cnat = pa.tile([128, ntc, D2], f32, tag="cnat")
                qnat = pa.tile([64, BL, D2], f32, tag="qnat")
                cT = pa.tile([128, 4, nt], f32, tag="cT")
                c2qT = pa.tile([128, 4, nt], f32, tag="c2qT")
                qT = pa.tile([128, 4, NQTOK], f32, tag="qT")
                qmT = pa.tile([128, 4, NQTOK], f32, tag="qmT")
                for ch in range(ntc):
                    nc.sync.dma_start(
                        out=cnat[:, ch, :], in_=c_d[128 * ch : 128 * (ch + 1), :]
                    )
                nc.sync.dma_start(
                    out=qnat[:], in_=q_d[:].rearrange("(b j) d -> j b d", j=J)
                )

                qnat2 = pa.tile([128, 4, D2], f32, tag="qnat2")
                for ch in range(4):
                    nc.sync.dma_start(
                        out=qnat2[:, ch, :], in_=q_d[128 * ch : 128 * (ch + 1), :]
                    )
                # transposes: cT[p=d%128, dc, tok], qT[p, dc, qtok]
                for ch in range(ntc):
                    for dc in range(4):
                        pt = psA.tile([128, 128], f32, tag="aps")
                        nc.tensor.transpose(
                            pt[:], cnat[:, ch, 128 * dc : 128 * (dc + 1)], eyesb[:]
                        )
                        nc.vector.tensor_copy(
                            cT[:, dc, 128 * ch : 128 * (ch + 1)], pt[:]
                        )
                for ch in range(4):
                    for dc in range(4):
                        pt = psA.tile([128, 128], f32, tag="aps")
                        nc.tensor.transpose(
                            pt[:], qnat2[:, ch, 128 * dc : 128 * (dc + 1)], eyesb[:]
                        )
                        nc.vector.tensor_copy(
                            qT[:, dc, 128 * ch : 128 * (ch + 1)], pt[:]
                        )

                # qmT = qT * wm + wc  (folds the c@wc rank-1 term into S)
                for dc in range(4):
                    nc.vector.tensor_scalar(
                        out=qmT[:, dc, :],
                        in0=qT[:, dc, :],
                        scalar1=wsb[:, 2, dc : dc + 1],
                        scalar2=wsb[:, 0, dc : dc + 1],
                        op0=ALU.mult,
                        op1=ALU.add,
                    )

                # v row: q @ wq  -> [1, NQTOK]
                vps = psA.tile([1, NQTOK], f32, tag="aps")
                for dc in range(4):
                    nc.tensor.matmul(
                        vps[:],
                        wsb[:, 1, dc : dc + 1],
                        qT[:, dc, :],
                        start=(dc == 0),
                        stop=(dc == 3),
                    )
                vrow = pa.tile([1, NQTOK], f32, tag="vrow")
                nc.vector.tensor_copy(vrow[:], vps[:])

                # S per (b, tchunk): S = cT.T @ qmT + v  -> softmax over J
                S2 = pa.tile([128, ntc, J], f32, tag="S2")
                nm = pa.tile([128, ntc], f32, tag="nm")
                sums = pa.tile([128, ntc], f32, tag="sums")
                rs = pa.tile([128, ntc], f32, tag="rs")
                AT = pa.tile([64, ntc, 128], f32, tag="AT")
                for b in range(BL):
                    for tch in range(tcpb):
                        col = b * tcpb + tch
                        sps = psA.tile([128, J], f32, tag="aps")
                        for dc in range(4):
                            st0 = b + 8 * 128 * tch
                            lhsT = cT[:, dc, st0 : st0 + 8 * 127 + 1 : 8]
                            nc.tensor.matmul(
                                sps[:],
                                lhsT,
                                qmT[:, dc, 64 * b : 64 * (b + 1)],
                                start=(dc == 0),
                                stop=False,
                            )
                        nc.tensor.matmul(
                            sps[:],
                            onesr[:],
                            vrow[0:1, 64 * b : 64 * (b + 1)],
                            start=False,
                            stop=True,
                        )
                        nc.vector.tensor_reduce(
                            nm[:, col : col + 1],
                            sps[:],
                            axis=mybir.AxisListType.X,
                            op=ALU.max,
                            negate=True,
                        )
                        nc.scalar.activation(
                            S2[:, col, :],
                            sps[:],
                            AF.Exp,
                            bias=nm[:, col : col + 1],
                            accum_out=sums[:, col : col + 1],
                        )
                nc.vector.reciprocal(rs[:, :], sums[:, :])
                for b in range(BL):
                    for tch in range(tcpb):
                        col = b * tcpb + tch
                        asc = pa2.tile([128, J], f32, tag="asc")
                        nc.vector.scalar_tensor_tensor(
                            asc[:],
                            S2[:, col, :],
                            rs[:, col : col + 1],
                            S2[:, col, :],
                            ALU.mult,
                            ALU.bypass,
                        )
                        atp = psA.tile([64, 128], f32, tag="aps")
                        nc.tensor.transpose(atp[:], asc[:], eyesb[:])
                        nc.vector.tensor_copy(AT[0:64, col, :], atp[:])

                # c2qT[d, tok] = q.T @ A.T  (per b)
                for b in range(BL):
                    for m in range(4):
                        cps = psA.tile([128, 128 * tcpb], f32, tag="aps")
                        lhsT = qnat[:, b, 128 * m : 128 * (m + 1)]
                        nc.tensor.matmul(
                            cps[:],
                            lhsT,
                            AT[0:64, b * tcpb : (b + 1) * tcpb, :],
                            start=True,
                            stop=True,
                        )
                        nc.vector.tensor_copy(
                            c2qT[:, m, b : nt : 8].rearrange(
                                "p (a c) -> p a c", a=tcpb
                            ),
                            cps[:].rearrange("p (a c) -> p a c", a=tcpb),
                        )

                # b_att = softmax_T(max_J S); mt holds max_J S = -nm
                mt = pa.tile([128, ntc], f32, tag="mt")
                nc.scalar.mul(mt[:, :], nm[:, :], -1.0)
                mtp = psA.tile([ntc, 128], f32, tag="aps")
                nc.tensor.transpose(mtp[:], mt[:, :], eyesb[:])
                mtT = pa.tile([ntc, 128], f32, tag="mtT")
                nc.vector.tensor_copy(mtT[:], mtp[:])

                s16 = pa.tile([ntc, 1], f32, tag="s16")
                nc.scalar.activation(
                    mtT[:], mtT[:], AF.Exp, bias=0.0, accum_out=s16[:]
                )
                wsc = pa.tile([ntc, 1], f32, tag="wsc")
                if tcpb > 1:
                    # pair-sum (b, tch) rows in the free dim of a 1-partition row
                    srp = psA.tile([1, ntc], f32, tag="aps", name="srp")
                    nc.tensor.transpose(srp[:], s16[:], eyesb[0:ntc, 0:ntc])
                    srow = pa.tile([1, ntc], f32, tag="srow")
                    nc.vector.tensor_copy(srow[:], srp[:])
                    zrow = pa.tile([1, BL], f32, tag="zrow")
                    nc.vector.tensor_tensor(
                        zrow[:], srow[0:1, 0:ntc:2], srow[0:1, 1:ntc:2], ALU.add
                    )
                    rrow = pa.tile([1, BL], f32, tag="rrow")
                    nc.vector.reciprocal(rrow[:], zrow[:])
                    r2row = pa.tile([1, ntc], f32, tag="r2row")
                    nc.vector.tensor_copy(r2row[0:1, 0:ntc:2], rrow[:])
                    nc.vector.tensor_copy(r2row[0:1, 1:ntc:2], rrow[:])
                    wsp = psA.tile([ntc, 1], f32, tag="aps", name="wsp")
                    nc.tensor.transpose(wsp[:], r2row[:], eyesb[0:1, 0:1])
                    nc.vector.tensor_copy(wsc[:], wsp[:])
                else:
                    nc.vector.reciprocal(wsc[:], s16[:])
                w16 = pa.tile([ntc, 128], f32, tag="w16")
                nc.vector.scalar_tensor_tensor(
                    w16[:], mtT[:], wsc[:], mtT[:], ALU.mult, ALU.bypass
                )
                wtp = psA.tile([128, ntc], f32, tag="aps")
                nc.tensor.transpose(wtp[:], w16[:], eyesb[0:ntc, 0:ntc])
                wT = pa.tile([128, ntc], f32, tag="wT")
                nc.vector.tensor_copy(wT[:], wtp[:])

                # q2c[b] = sum_t w[b,t] c[b,t,:], with t-major tokens:
                # masked contraction over all tokens. wmask[p, ch, b] = w(token)
                # iff token%8==b. Token tok=128ch+p, t=tok//8; w16 flat index
                # = 256*b + t (tcpb=2) i.e. tcpb*128*b + t.
                nc.sync.dma_start(out=wb_d[0 : ntc, :], in_=w16[:, :])
                wmask = pa.tile([128, ntc, BL], f32, tag="wmask")
                nc.vector.memset(wmask[:], 0.0)
                wbflat = wb_d[:].rearrange("a c -> (a c)")
                for b in range(BL):
                    nc.sync.dma_start(
                        out=wmask[b : 121 + b : 8, :, b],
                        in_=wbflat[
                            tcpb * 128 * b : tcpb * 128 * (b + 1)
                        ].rearrange("(ch j) -> j ch", j=16),
                    )
                q2ps = psA.tile([BL, D2], f32, tag="q2ps")
                for ch in range(ntc):
                    nc.tensor.matmul(
                        q2ps[:],
                        wmask[:, ch, :],
                        cnat[:, ch, :],
                        start=(ch == 0),
                        stop=(ch == ntc - 1),
                    )
                q2c8 = pa.tile([BL, D2], f32, tag="q2c8")
                nc.vector.tensor_copy(q2c8[:], q2ps[:])
                q2cT = pa.tile([128, 4, BL], f32, tag="q2cT")
                for dc in range(4):
                    qtp = psA.tile([128, BL], f32, tag="aps")
                    nc.tensor.transpose(
                        qtp[:], q2c8[:, 128 * dc : 128 * (dc + 1)], eyesb[0:BL, 0:BL]
                    )
                    nc.vector.tensor_copy(q2cT[:, dc, :], qtp[:])

                # emit G.T = [c; c2q; c*c2q; c*q2c].T to DRAM
                for dc in range(4):
                    nc.sync.dma_start(out=gt_d[dc, :, :], in_=cT[:, dc, :])
                    nc.sync.dma_start(out=gt_d[4 + dc, :, :], in_=c2qT[:, dc, :])
                for dc in range(4):
                    nc.vector.tensor_tensor(
                        c2qT[:, dc, :], cT[:, dc, :], c2qT[:, dc, :], ALU.mult
                    )
                    nc.sync.dma_start(out=gt_d[8 + dc, :, :], in_=c2qT[:, dc, :])
                for dc in range(4):
                    cview = cT[:, dc, :].rearrange("p (t b) -> p t b", b=BL)
                    bview = q2cT[:, dc, :].rearrange("p (o b) -> p o b", o=1).broadcast_to(
                        [128, t_len, BL]
                    )
                    nc.vector.tensor_tensor(cview, cview, bview, ALU.mult)
                    nc.sync.dma_start(out=gt_d[12 + dc, :, :], in_=cT[:, dc, :])

            # ===============================================================
            # PHASE B: 4 biGRU layers (gi batched matmul + latency-opt scan)
            # ===============================================================
            with (
                tc.tile_pool(name="main", bufs=1) as pm,
                tc.tile_pool(name="wst", bufs=1) as pw,
                tc.tile_pool(name="gst", bufs=3) as pg,
                tc.tile_pool(name="scr", bufs=8) as pscr,
            ):
                gi = pm.tile([128, t_len, 6, 16], bf16, tag="gi")
                yT_f = pm.tile([128, 2, nt], f32r, tag="yTf", name="yT_f")
                yT_b = pm.tile([128, 2, nt], f32r, tag="yTb", name="yT_b")
                yTd = [yT_f, yT_b]
                h0f = pm.tile([128, 2, BL], bf16, tag="h0f")
                h0b = pm.tile([128, 2, BL], bf16, tag="h0b")
                nc.vector.memset(h0f[:], 0.0)
                nc.vector.memset(h0b[:], 0.0)

                SL = min(512, nt)
                nsl = nt // SL
                slt = SL // BL

                prev_stage = {}

                def order(key, inst):
                    if key in prev_stage:
                        add_dep_helper(
                            inst.ins, prev_stage[key], sync=False,
                            reason="scan stagger order",
                        )
                    prev_stage[key] = inst.ins
                    return inst

                for layer in range(4):
                    nk = NKL[layer]
                    # ---- gi build: weights resident per half, psum 6 banks
                    with tc.tile_pool(name="gips", bufs=1, space="PSUM") as pgi:
                        gacc = [
                            pgi.tile([128, SL], f32, tag=f"gacc{i}", name=f"gacc{i}")
                            for i in range(6)
                        ]
                        for half in range(2):
                            wihs = pw.tile(
                                [128, nk, 6, 128], f32r, tag="wihs", name="wihs"
                            )
                            nc.sync.dma_start(
                                out=wihs[:],
                                in_=wih_d[layer][
                                    :, 768 * half : 768 * (half + 1)
                                ]
                                .rearrange("(ki p) (mm j) -> p ki mm j", p=128, j=128)
                                .bitcast(f32r),
                            )
                            for s in range(nsl):
                                for ki in range(nk):
                                    if layer == 0 or (layer == 2 and ki < 16):
                                        rt = pg.tile([128, SL], f32r, tag="gs")
                                        nc.sync.dma_start(
                                            out=rt[:],
                                            in_=gt_d[
                                                ki, :, SL * s : SL * (s + 1)
                                            ].bitcast(f32r),
                                        )
                                        rhs = rt[:]
                                    else:
                                        kc = ki - 16 if layer == 2 else ki
                                        ysrc = yTd[kc // 2]
                                        rhs = ysrc[:, kc % 2, SL * s : SL * (s + 1)]
                                    for mm in range(6):
                                        nc.tensor.matmul(
                                            gacc[mm][:],
                                            wihs[:, ki, mm, :],
                                            rhs,
                                            start=(ki == 0),
                                            stop=(ki == nk - 1),
                                        )
                                for mm in range(6):
                                    gslice = gi[
                                        :,
                                        slt * s : slt * (s + 1),
                                        mm,
                                        8 * half : 8 * (half + 1),
                                    ]
                                    nc.scalar.activation(
                                        gslice,
                                        gacc[mm][:].rearrange("p (a b) -> p a b", b=8),
                                        AF.Identity,
                                        bias=gbsb[:, layer, half, mm : mm + 1],
                                    )

                    # ---- scan: 2 staggered chains (fwd, bwd),
                    # gates-on-partition, eye-matmul psum preloads
                    with (
                        tc.tile_pool(name="psf", bufs=2, space="PSUM") as ppf,
                        tc.tile_pool(name="psb", bufs=2, space="PSUM") as ppb,
                    ):
                        hprev = [[h0f[:]], [h0b[:]]]
                        hmat = [h0f[:], h0b[:]]
                        Pcur = [None, None]

                        def emit_pre(d, step):
                            pp = ppf if d == 0 else ppb
                            t = step if d == 0 else t_len - 1 - step
                            cs = 8 * d
                            Pfull = pp.tile(
                                [128, 64, 8], f32, tag=f"P{d}", name=f"Pfull{d}"
                            )
                            P = Pfull[:, 0:6, :]
                            order("PE", nc.tensor.matmul(
                                P[:, 0:4, :], eyeb[:], gi[:, t, 0:4, cs : cs + 8],
                                start=True, stop=False, skip_group_check=True,
                            ))
                            order("PE", nc.tensor.matmul(
                                P[:, 4:6, :], eyeb[:], bhnt[:, layer, :, cs : cs + 8],
                                start=False, stop=False, skip_group_check=True,
                            ))
                            Pcur[d] = P

                        def emit(d, step):
                            t = step if d == 0 else t_len - 1 - step
                            cs = 8 * d
                            P = Pcur[d]
                            for m in range(6):
                                for kc in range(2):
                                    for hpart in hprev[d]:
                                        order("PE", nc.tensor.matmul(
                                            P[:, m, :],
                                            whhs[:, layer, d, kc, m, :],
                                            hpart[:, kc, :],
                                            start=False,
                                            stop=(kc == 1 and hpart is hprev[d][-1]),
                                            skip_group_check=True,
                                        ))
                            if step + 1 < t_len:
                                emit_pre(d, step + 1)
                            rz = pscr.tile(
                                [128, 4, 8], bf16, tag=f"rz{d}", name=f"rz{d}"
                            )
                            order("ACT", nc.scalar.activation(
                                rz[:], P[:, 0:4, :], AF.Sigmoid
                            ))
                            ntl = pscr.tile(
                                [128, 2, 8], f32, tag=f"ntl{d}", name=f"ntl{d}"
                            )
                            order("DVE", nc.vector.tensor_tensor(
                                ntl[:], P[:, 4:6, :], rz[:, 0:2, :], ALU.mult
                            ))
                            order("DVE", nc.vector.tensor_tensor(
                                ntl[:], ntl[:], gi[:, t, 4:6, cs : cs + 8], ALU.add
                            ))
                            nn = pscr.tile(
                                [128, 2, 8], bf16, tag=f"nn{d}", name=f"nn{d}"
                            )
                            order("ACT", nc.scalar.activation(nn[:], ntl[:], AF.Tanh))
                            # h' = (1-z)*n + z*h; next matmul streams zh and u
                            # as two moving operands so h' is off-path
                            zh = pscr.tile(
                                [128, 2, 8], bf16, tag=f"zh{d}", name=f"zh{d}"
                            )
                            order("Pool", nc.gpsimd.tensor_tensor(
                                zh[:], rz[:, 2:4, :], hmat[d], ALU.mult
                            ))
                            zc = pscr.tile(
                                [128, 2, 8], bf16, tag=f"zc{d}", name=f"zc{d}"
                            )
                            order("Pool", nc.gpsimd.tensor_scalar(
                                out=zc[:], in0=rz[:, 2:4, :], scalar1=-1.0,
                                scalar2=1.0, op0=ALU.mult, op1=ALU.add,
                            ))
                            u = pscr.tile(
                                [128, 2, 8], bf16, tag=f"u{d}", name=f"u{d}"
                            )
                            order("DVE", nc.vector.tensor_tensor(
                                u[:], zc[:], nn[:], ALU.mult
                            ))
                            hnew = yTd[d][:, :, BL * t : BL * (t + 1)]
                            order("Pool", nc.gpsimd.tensor_tensor(
                                hnew, zh[:], u[:], ALU.add
                            ))
                            hprev[d] = [zh[:], u[:]]
                            hmat[d] = hnew

                        emit_pre(0, 0)
                        emit_pre(1, 0)
                        for step in range(t_len):
                            emit(0, step)
                            emit(1, step)
                        if layer < 3:
                            # reset h0 for the next layer (cheap, off-path)
                            nc.vector.memset(h0f[:], 0.0)
                            nc.vector.memset(h0b[:], 0.0)

                # output: [hb, hf] per sequence; finals live in yT slices
                for hh, dd in ((0, 1), (1, 0)):
                    src_y = yTd[dd]
                    tslice = (
                        slice(0, BL) if dd == 1 else slice(BL * (t_len - 1), BL * t_len)
                    )
                    for chh in range(2):
                        c0 = 256 * hh + 128 * chh
                        ov = out_d[:, c0 : c0 + 128].rearrange("b p -> p b")
                        nc.sync.dma_start(
                            out=ov, in_=src_y[:, chh, tslice].bitcast(f32)
                        )

    return nc


# ---------------------------------------------------------------------------
# host-side weight prep
# ---------------------------------------------------------------------------
def _prep_weights(inputs):
    import ml_dtypes
    bf = ml_dtypes.bfloat16
    names = ["mod0", "mod1", "rep0", "rep1"]
    wih = []
    whhs = np.empty((128, 4, 2, 2, 6, 128), np.float32)
    gb = np.empty((128, 4, 2, 6), np.float32)
    bhnt = np.empty((128, 4, 2, 16), np.float32)
    for layer, nm in enumerate(names):
        Wih = np.asarray(inputs[f"{nm}_Wih"], np.float32)   # [2, 768, in]
        Whh = np.asarray(inputs[f"{nm}_Whh"], np.float32)   # [2, 768, 256]
        bb = np.asarray(inputs[f"{nm}_b"], np.float32)      # [2, 2, 768]
        wih.append(
            np.ascontiguousarray(np.concatenate([Wih[0].T, Wih[1].T], axis=1))
        )
        for d in range(2):
            # whhs[p, l, d, kc, m, j] = Whh[d].T[kc*128+p, m*128+j]
            WT = Whh[d].T.reshape(2, 128, 6, 128)
            whhs[:, layer, d] = WT.transpose(1, 0, 2, 3)
            vec = bb[d, 0] + np.concatenate([bb[d, 1][:D2], np.zeros(D, np.float32)])
            gb[:, layer, d, :] = vec.reshape(6, 128).T
            bhnt[:, layer, :, 8 * d : 8 * d + 8] = (
                bb[d, 1][D2:].reshape(2, 128).T[:, :, None]
            )
    return wih, whhs.astype(bf), gb, bhnt.astype(bf)


_PROG = None


def kernel(**inputs):
    global _PROG
    if _PROG is None:
        _PROG = build_program()
    nc = _PROG

    import ml_dtypes

    wih, whhs, gb, bhnt = _prep_weights(inputs)
    ws = np.asarray(inputs["Ws"], np.float32).reshape(3, D2)
    eye = np.eye(128, dtype=np.float32)
    c_all = np.asarray(inputs["embd_context"], np.float32)
    q_all = np.asarray(inputs["embd_query"], np.float32)

    shared = {
        "eye": eye,
        "eyeb": np.eye(128, dtype=ml_dtypes.bfloat16),
        "wsplit": np.ascontiguousarray(ws),
        "whhs": whhs,
        "gbias": gb,
        "bhnt": bhnt,
    }
    for layer in range(4):
        shared[f"wih{layer}"] = wih[layer]

    in_maps = []
    for i in range(NCORES):
        ci = c_all[BL * i : BL * (i + 1)]           # [8, 256, 512]
        c_tm = np.ascontiguousarray(
            ci.transpose(1, 0, 2).reshape(T * BL, D2)
        )
        qi = np.ascontiguousarray(
            q_all[BL * i : BL * (i + 1)].reshape(NQTOK, D2)
        )
        m = dict(shared)
        m["c"] = c_tm
        m["q"] = qi
        in_maps.append(m)

    res = run_bass_kernel_spmd(nc, in_maps, core_ids=list(range(NCORES)))
    out = np.concatenate([res.results[i]["out"] for i in range(NCORES)], axis=0)
    return np.ascontiguousarray(out.astype(np.float32))



# revision 20
# speedup vs baseline: 3.0808x; 2.0961x over previous
"""Trainium2 Bass kernel for nn_BiDAF_Wemb.

Data-parallel over batch: 8 NeuronCores x 8 sequences each. Per core:
  attention (BiDAF) -> G.T in DRAM -> per layer: gi = x @ Wih.T (batched,
  fp32r) -> bidirectional GRU scan with h-stationary streaming matmuls,
  a 4-instruction DVE block-transpose into gates-on-partition layout,
  and fused elementwise gate math split across DVE/ACT/GPSIMD.

Self-contained: hardcodes all shapes; builds the Bass program on first call.
"""

import numpy as np

import bass_rust
import concourse.bass as bass
import concourse.mybir as mybir
import concourse.tile as tile_mod
from concourse.tile import TileContext
from concourse.tile_rust import add_dep_helper
from concourse.bass_utils import run_bass_kernel_spmd

f32 = mybir.dt.float32
f32r = mybir.dt.float32r
bf16 = mybir.dt.bfloat16
AF = mybir.ActivationFunctionType
ALU = mybir.AluOpType

B, T, J, D = 64, 256, 64, 256
D2, H3 = 2 * D, 3 * D            # 512, 768
NCORES = 8
BL = B // NCORES                 # 8 sequences per core
NTOK = BL * T                    # 2048 tokens per core (t-major: col = t*BL + b)
NQTOK = BL * J                   # 512 query tokens (b-major: row = b*J + j)
IN_L = [8 * D, D2, 10 * D, D2]   # gi input widths per layer
NKL = [x // 128 for x in IN_L]   # K-chunks per layer: 16, 4, 20, 4


# ---------------------------------------------------------------------------
# toolchain patches: walrus in this container rejects >1 embedded sync-wait
# per instruction; split extras onto same-engine NoOp carriers.
# ---------------------------------------------------------------------------
def _patch_tile():
    if getattr(tile_mod.TileContext, "_bidaf_patched", False):
        return
    LIMIT = 1
    counter = [0]
    orig_lower = tile_mod.TileContext._lower_ordered_insts

    def split_list(nc, insts):
        # Walk in scheduled order tracking per-semaphore emitted updates.
        # For multi-wait instructions keep the binding wait (min slack)
        # embedded; other waits ride on ENGINE_NOP carriers, which wait in
        # the engine wait-queue rather than blocking the sequencer.
        eng_map = {
            mybir.EngineType.PE: nc.tensor,
            mybir.EngineType.DVE: nc.vector,
            mybir.EngineType.Activation: nc.scalar,
            mybir.EngineType.Pool: nc.gpsimd,
            mybir.EngineType.SP: nc.sync,
        }
        nop_op = nc.isa.Opcode.NEURON_ISA_TPB_OPCODE_ENGINE_NOP
        counts = {}
        out = []
        for inst in insts:
            lim = 1
            si = inst.sync_info
            waits = list(si.on_wait) if si is not None else []
            if len(waits) > lim:
                def slack(w):
                    v = w.wait_value if isinstance(w.wait_value, int) else 0
                    return counts.get((w.sync_type, w.id), 0) - v
                waits.sort(key=slack)
                rest = waits[lim:]
                for i in range(0, len(rest), lim):
                    counter[0] += 1
                    nop = mybir.InstNoOp(name=f"WS-{counter[0]}", engine=inst.engine)
                    nop.sync_info = bass_rust.SyncInfo(
                        on_wait=rest[i : i + lim], on_update=[]
                    )
                    out.append(nop)
                si.on_wait = waits[:lim]
                inst.sync_info = si
            if si is not None:
                for u in si.on_update:
                    v = u.update_value if isinstance(u.update_value, int) else 0
                    k = (u.sync_type, u.id)
                    counts[k] = counts.get(k, 0) + v
            out.append(inst)
        return out

    def patched_lower(self, ordered):
        for k in list(ordered.keys()):
            ordered[k] = split_list(self.nc, ordered[k])
        return orig_lower(self, ordered)

    def patched_drain(self, tick_clock, wait_clock):
        nc = self.nc
        drain_inst = nc.sync.drain()
        wait_clock.add_sem_waits(
            drain_inst.ins, tile_mod.ScopedClock({None: tick_clock.global_clock})
        )
        si = drain_inst.ins.sync_info
        if si is not None and len(si.on_wait) > LIMIT:
            waits = list(si.on_wait)
            si.on_wait = waits[:LIMIT]
            drain_inst.ins.sync_info = si
            for i in range(LIMIT, len(waits), LIMIT):
                extra = nc.sync.drain()
                extra.ins.sync_info = bass_rust.SyncInfo(
                    on_wait=waits[i : i + LIMIT], on_update=[]
                )
        nc.all_engine_barrier()
        popped = nc._tile_sem_poison_stack.pop()
        assert popped is self._sem_poison
        nc.clear_and_free_semaphores(list(self.sems.allocated().values()))
        nc.all_engine_barrier()

    tile_mod.TileContext._lower_ordered_insts = patched_lower
    tile_mod.TileContext._drain_and_barrier = patched_drain
    tile_mod.TileContext._bidaf_patched = True


# ---------------------------------------------------------------------------
# program builder
# ---------------------------------------------------------------------------
def build_program(t_len=T, debug=False):
    _patch_tile()
    nt = BL * t_len          # tokens
    ntc = nt // 128          # 128-token chunks
    tcpb = t_len // 128      # t-chunks per sequence (2 at full size)

    nc = bass.Bass("TRN2", target_bir_lowering=False, debug=False)

    c_d = nc.dram_tensor("c", [nt, D2], f32, kind="ExternalInput")
    q_d = nc.dram_tensor("q", [NQTOK, D2], f32, kind="ExternalInput")
    eye_d = nc.dram_tensor("eye", [128, 128], f32, kind="ExternalInput")
    ws_d = nc.dram_tensor("wsplit", [3, D2], f32, kind="ExternalInput")
    wih_d = [
        nc.dram_tensor(f"wih{layer}", [IN_L[layer], 2 * H3], f32, kind="ExternalInput")
        for layer in range(4)
    ]
    whh_d = nc.dram_tensor("whhs", [128, 4, 2, 2, 6, 128], bf16, kind="ExternalInput")
    gb_d = nc.dram_tensor("gbias", [128, 4, 2, 6], f32, kind="ExternalInput")
    bhnt_d = nc.dram_tensor("bhnt", [128, 4, 2, 16], bf16, kind="ExternalInput")
    eyeb_d = nc.dram_tensor("eyeb", [128, 128], bf16, kind="ExternalInput")
    wb_d = nc.dram_tensor("wbounce", [BL * 2, 128], f32)
    out_d = nc.dram_tensor("out", [BL, D2], f32, kind="ExternalOutput")
    gt_d = nc.dram_tensor("GT", [16, 128, nt], f32, kind="Internal")
    gi2_d = nc.dram_tensor("GI2", [128, 2, 6, nt], bf16, kind="Internal")

    with TileContext(nc) as tc:
        with tc.tile_pool(name="const", bufs=1) as pc:
            eyesb = pc.tile([128, 128], f32, tag="eye")
            eyeb = pc.tile([128, 128], bf16, tag="eyeb")
            wsb = pc.tile([128, 3, 4], f32, tag="wsb")        # [p, (wc,wq,wm), dchunk]
            gbsb = pc.tile([128, 4, 2, 6], f32, tag="gbsb")
            bhnt = pc.tile([128, 4, 2, 16], bf16, tag="bhnt")
            whhs = pc.tile([128, 4, 2, 2, 6, 128], bf16, tag="whhs")
            onesr = pc.tile([1, 128], f32, tag="onesr")
            nc.sync.dma_start(out=eyesb[:], in_=eye_d[:])
            nc.sync.dma_start(out=eyeb[:], in_=eyeb_d[:])
            nc.sync.dma_start(
                out=wsb[:], in_=ws_d[:].rearrange("i (dc p) -> p i dc", p=128)
            )
            nc.sync.dma_start(out=gbsb[:], in_=gb_d[:])
            nc.sync.dma_start(out=bhnt[:], in_=bhnt_d[:])
            nc.sync.dma_start(out=whhs[:], in_=whh_d[:])
            nc.vector.memset(onesr[:], 1.0)

            # ===============================================================
            # PHASE A: attention -> G.T (DRAM)
            # ===============================================================
            with (
                tc.tile_pool(name="attn", bufs=1) as pa,
                tc.tile_pool(name="attn2", bufs=2) as pa2,
                tc.tile_pool(name="psA", bufs=4, space="PSUM") as psA,
            ):
                cnat = pa.tile([128, ntc, D2], f32, tag="cnat")
                qnat = pa.tile([64, BL, D2], f32, tag="qnat")
                cT = pa.tile([128, 4, nt], f32, tag="cT")
                c2qT = pa.tile([128, 4, nt], f32, tag="c2qT")
                qT = pa.tile([128, 4, NQTOK], f32, tag="qT")
                qmT = pa.tile([128, 4, NQTOK], f32, tag="qmT")
                for ch in range(ntc):
                    nc.sync.dma_start(
                        out=cnat[:, ch, :], in_=c_d[128 * ch : 128 * (ch + 1), :]
                    )
                nc.sync.dma_start(
                    out=qnat[:], in_=q_d[:].rearrange("(b j) d -> j b d", j=J)
                )

                qnat2 = pa.tile([128, 4, D2], f32, tag="qnat2")
                for ch in range(4):
                    nc.sync.dma_start(
                        out=qnat2[:, ch, :], in_=q_d[128 * ch : 128 * (ch + 1), :]
                    )
                # transposes: cT[p=d%128, dc, tok], qT[p, dc, qtok]
                for ch in range(ntc):
                    for dc in range(4):
                        pt = psA.tile([128, 128], f32, tag="aps")
                        nc.tensor.transpose(
                            pt[:], cnat[:, ch, 128 * dc : 128 * (dc + 1)], eyesb[:]
                        )
                        nc.vector.tensor_copy(
                            cT[:, dc, 128 * ch : 128 * (ch + 1)], pt[:]
                        )
                for ch in range(4):
                    for dc in range(4):
                        pt = psA.tile([128, 128], f32, tag="aps")
                        nc.tensor.transpose(
                            pt[:], qnat2[:, ch, 128 * dc : 128 * (dc + 1)], eyesb[:]
                        )
                        nc.vector.tensor_copy(
                            qT[:, dc, 128 * ch : 128 * (ch + 1)], pt[:]
                        )

                # qmT = qT * wm + wc  (folds the c@wc rank-1 term into S)
                for dc in range(4):
                    nc.vector.tensor_scalar(
                        out=qmT[:, dc, :],
                        in0=qT[:, dc, :],
                        scalar1=wsb[:, 2, dc : dc + 1],
                        scalar2=wsb[:, 0, dc : dc + 1],
                        op0=ALU.mult,
                        op1=ALU.add,
                    )

                # v row: q @ wq  -> [1, NQTOK]
                vps = psA.tile([1, NQTOK], f32, tag="aps")
                for dc in range(4):
                    nc.tensor.matmul(
                        vps[:],
                        wsb[:, 1, dc : dc + 1],
                        qT[:, dc, :],
                        start=(dc == 0),
                        stop=(dc == 3),
                    )
                vrow = pa.tile([1, NQTOK], f32, tag="vrow")
                nc.vector.tensor_copy(vrow[:], vps[:])

                # S per (b, tchunk): S = cT.T @ qmT + v  -> softmax over J
                S2 = pa.tile([128, ntc, J], f32, tag="S2")
                nm = pa.tile([128, ntc], f32, tag="nm")
                sums = pa.tile([128, ntc], f32, tag="sums")
                rs = pa.tile([128, ntc], f32, tag="rs")
                AT = pa.tile([64, ntc, 128], f32, tag="AT")
                for b in range(BL):
                    for tch in range(tcpb):
                        col = b * tcpb + tch
                        sps = psA.tile([128, J], f32, tag="aps")
                        for dc in range(4):
                            st0 = b + 8 * 128 * tch
                            lhsT = cT[:, dc, st0 : st0 + 8 * 127 + 1 : 8]
                            nc.tensor.matmul(
                                sps[:],
                                lhsT,
                                qmT[:, dc, 64 * b : 64 * (b + 1)],
                                start=(dc == 0),
                                stop=False,
                            )
                        nc.tensor.matmul(
                            sps[:],
                            onesr[:],
                            vrow[0:1, 64 * b : 64 * (b + 1)],
                            start=False,
                            stop=True,
                        )
                        nc.vector.tensor_reduce(
                            nm[:, col : col + 1],
                            sps[:],
                            axis=mybir.AxisListType.X,
                            op=ALU.max,
                            negate=True,
                        )
                        nc.scalar.activation(
                            S2[:, col, :],
                            sps[:],
                            AF.Exp,
                            bias=nm[:, col : col + 1],
                            accum_out=sums[:, col : col + 1],
                        )
                nc.vector.reciprocal(rs[:, :], sums[:, :])
                for b in range(BL):
                    for tch in range(tcpb):
                        col = b * tcpb + tch
                        asc = pa2.tile([128, J], f32, tag="asc")
                        nc.vector.scalar_tensor_tensor(
                            asc[:],
                            S2[:, col, :],
                            rs[:, col : col + 1],
                            S2[:, col, :],
                            ALU.mult,
                            ALU.bypass,
                        )
                        atp = psA.tile([64, 128], f32, tag="aps")
                        nc.tensor.transpose(atp[:], asc[:], eyesb[:])
                        nc.vector.tensor_copy(AT[0:64, col, :], atp[:])

                # c2qT[d, tok] = q.T @ A.T  (per b)
                for b in range(BL):
                    for m in range(4):
                        cps = psA.tile([128, 128 * tcpb], f32, tag="aps")
                        lhsT = qnat[:, b, 128 * m : 128 * (m + 1)]
                        nc.tensor.matmul(
                            cps[:],
                            lhsT,
                            AT[0:64, b * tcpb : (b + 1) * tcpb, :],
                            start=True,
                            stop=True,
                        )
                        nc.vector.tensor_copy(
                            c2qT[:, m, b : nt : 8].rearrange(
                                "p (a c) -> p a c", a=tcpb
                            ),
                            cps[:].rearrange("p (a c) -> p a c", a=tcpb),
                        )

                # b_att = softmax_T(max_J S); mt holds max_J S = -nm
                mt = pa.tile([128, ntc], f32, tag="mt")
                nc.scalar.mul(mt[:, :], nm[:, :], -1.0)
                mtp = psA.tile([ntc, 128], f32, tag="aps")
                nc.tensor.transpose(mtp[:], mt[:, :], eyesb[:])
                mtT = pa.tile([ntc, 128], f32, tag="mtT")
                nc.vector.tensor_copy(mtT[:], mtp[:])

                s16 = pa.tile([ntc, 1], f32, tag="s16")
                nc.scalar.activation(
                    mtT[:], mtT[:], AF.Exp, bias=0.0, accum_out=s16[:]
                )
                wsc = pa.tile([ntc, 1], f32, tag="wsc")
                if tcpb > 1:
                    # pair-sum (b, tch) rows in the free dim of a 1-partition row
                    srp = psA.tile([1, ntc], f32, tag="aps", name="srp")
                    nc.tensor.transpose(srp[:], s16[:], eyesb[0:ntc, 0:ntc])
                    srow = pa.tile([1, ntc], f32, tag="srow")
                    nc.vector.tensor_copy(srow[:], srp[:])
                    zrow = pa.tile([1, BL], f32, tag="zrow")
                    nc.vector.tensor_tensor(
                        zrow[:], srow[0:1, 0:ntc:2], srow[0:1, 1:ntc:2], ALU.add
                    )
                    rrow = pa.tile([1, BL], f32, tag="rrow")
                    nc.vector.reciprocal(rrow[:], zrow[:])
                    r2row = pa.tile([1, ntc], f32, tag="r2row")
                    nc.vector.tensor_copy(r2row[0:1, 0:ntc:2], rrow[:])
                    nc.vector.tensor_copy(r2row[0:1, 1:ntc:2], rrow[:])
                    wsp = psA.tile([ntc, 1], f32, tag="aps", name="wsp")
                    nc.tensor.transpose(wsp[:], r2row[:], eyesb[0:1, 0:1])
                    nc.vector.tensor_copy(wsc[:], wsp[:])
                else:
                    nc.vector.reciprocal(wsc[:], s16[:])
                w16 = pa.tile([ntc, 128], f32, tag="w16")
                nc.vector.scalar_tensor_tensor(
                    w16[:], mtT[:], wsc[:], mtT[:], ALU.mult, ALU.bypass
                )
                wtp = psA.tile([128, ntc], f32, tag="aps")
                nc.tensor.transpose(wtp[:], w16[:], eyesb[0:ntc, 0:ntc])
                wT = pa.tile([128, ntc], f32, tag="wT")
                nc.vector.tensor_copy(wT[:], wtp[:])

                # q2c[b] = sum_t w[b,t] c[b,t,:], with t-major tokens:
                # masked contraction over all tokens. wmask[p, ch, b] = w(token)
                # iff token%8==b. Token tok=128ch+p, t=tok//8; w16 flat index
                # = 256*b + t (tcpb=2) i.e. tcpb*128*b + t.
                nc.sync.dma_start(out=wb_d[0 : ntc, :], in_=w16[:, :])
                wmask = pa.tile([128, ntc, BL], f32, tag="wmask")
                nc.vector.memset(wmask[:], 0.0)
                wbflat = wb_d[:].rearrange("a c -> (a c)")
                for b in range(BL):
                    nc.sync.dma_start(
                        out=wmask[b : 121 + b : 8, :, b],
                        in_=wbflat[
                            tcpb * 128 * b : tcpb * 128 * (b + 1)
                        ].rearrange("(ch j) -> j ch", j=16),
                    )
                q2ps = psA.tile([BL, D2], f32, tag="q2ps")
                for ch in range(ntc):
                    nc.tensor.matmul(
                        q2ps[:],
                        wmask[:, ch, :],
                        cnat[:, ch, :],
                        start=(ch == 0),
                        stop=(ch == ntc - 1),
                    )
                q2c8 = pa.tile([BL, D2], f32, tag="q2c8")
                nc.vector.tensor_copy(q2c8[:], q2ps[:])
                q2cT = pa.tile([128, 4, BL], f32, tag="q2cT")
                for dc in range(4):
                    qtp = psA.tile([128, BL], f32, tag="aps")
                    nc.tensor.transpose(
                        qtp[:], q2c8[:, 128 * dc : 128 * (dc + 1)], eyesb[0:BL, 0:BL]
                    )
                    nc.vector.tensor_copy(q2cT[:, dc, :], qtp[:])

                # emit G.T = [c; c2q; c*c2q; c*q2c].T to DRAM
                for dc in range(4):
                    nc.sync.dma_start(out=gt_d[dc, :, :], in_=cT[:, dc, :])
                    nc.sync.dma_start(out=gt_d[4 + dc, :, :], in_=c2qT[:, dc, :])
                for dc in range(4):
                    nc.vector.tensor_tensor(
                        c2qT[:, dc, :], cT[:, dc, :], c2qT[:, dc, :], ALU.mult
                    )
                    nc.sync.dma_start(out=gt_d[8 + dc, :, :], in_=c2qT[:, dc, :])
                for dc in range(4):
                    cview = cT[:, dc, :].rearrange("p (t b) -> p t b", b=BL)
                    bview = q2cT[:, dc, :].rearrange("p (o b) -> p o b", o=1).broadcast_to(
                        [128, t_len, BL]
                    )
                    nc.vector.tensor_tensor(cview, cview, bview, ALU.mult)
                    nc.sync.dma_start(out=gt_d[12 + dc, :, :], in_=cT[:, dc, :])

            # ===============================================================
            # PHASE B: 4 biGRU layers (gi batched matmul + latency-opt scan)
            # ===============================================================
            with (
                tc.tile_pool(name="main", bufs=1) as pm,
                tc.tile_pool(name="wst", bufs=1) as pw,
                tc.tile_pool(name="gst", bufs=3) as pg,
                tc.tile_pool(name="scr", bufs=8) as pscr,
            ):
                gi = pm.tile([128, t_len, 6, 16], bf16, tag="gi")
                yT_f = pm.tile([128, 2, nt], f32r, tag="yTf", name="yT_f")
                yT_b = pm.tile([128, 2, nt], f32r, tag="yTb", name="yT_b")
                yTd = [yT_f, yT_b]
                h0f = pm.tile([128, 2, BL], bf16, tag="h0f")
                h0b = pm.tile([128, 2, BL], bf16, tag="h0b")
                nc.vector.memset(h0f[:], 0.0)
                nc.vector.memset(h0b[:], 0.0)

                SL = min(512, nt)
                nsl = nt // SL
                slt = SL // BL

                prev_stage = {}
                l2g_work = []  # deferred layer-2 G-part gi matmuls

                def order(key, inst):
                    if key in prev_stage:
                        add_dep_helper(
                            inst.ins, prev_stage[key], sync=False,
                            reason="scan stagger order",
                        )
                    prev_stage[key] = inst.ins
                    return inst

                for layer in range(4):
                    nk = NKL[layer]
                    # ---- gi build: weights resident per half, psum 6 banks
                    with tc.tile_pool(name="gips", bufs=1, space="PSUM") as pgi:
                        gacc = [
                            pgi.tile([128, SL], f32, tag=f"gacc{i}", name=f"gacc{i}")
                            for i in range(6)
                        ]
                        for half in range(2):
                            wihs = pw.tile(
                                [128, nk, 6, 128], f32r, tag="wihs", name="wihs"
                            )
                            wsrc = wih_d[layer][:, 768 * half : 768 * (half + 1)]
                            if layer == 2:
                                wsrc = wih_d[2][128 * 16 :, 768 * half : 768 * (half + 1)]
                                wdst = wihs[:, 16:, :, :]
                            else:
                                wdst = wihs[:]
                            nc.sync.dma_start(
                                out=wdst,
                                in_=wsrc.rearrange(
                                    "(ki p) (mm j) -> p ki mm j", p=128, j=128
                                ).bitcast(f32r),
                            )
                            for s in range(nsl):
                                kis = range(16, nk) if layer == 2 else range(nk)
                                if layer == 2:
                                    # preload psum with the G-part accumulated
                                    # during scan 0 (eye-matmul from spill)
                                    g2t = pg.tile(
                                        [128, 6, SL], bf16, tag="g2t", name="g2t"
                                    )
                                    nc.sync.dma_start(
                                        out=g2t[:],
                                        in_=gi2_d[:, half, :, SL * s : SL * (s + 1)],
                                    )
                                    for mm in range(6):
                                        nc.tensor.matmul(
                                            gacc[mm][:],
                                            eyeb[:],
                                            g2t[:, mm, :],
                                            start=True,
                                            stop=False,
                                            skip_group_check=True,
                                        )
                                for ki in kis:
                                    if layer == 0:
                                        rt = pg.tile([128, SL], f32r, tag="gs")
                                        nc.sync.dma_start(
                                            out=rt[:],
                                            in_=gt_d[
                                                ki, :, SL * s : SL * (s + 1)
                                            ].bitcast(f32r),
                                        )
                                        rhs = rt[:]
                                    else:
                                        kc = ki - 16 if layer == 2 else ki
                                        ysrc = yTd[kc // 2]
                                        rhs = ysrc[:, kc % 2, SL * s : SL * (s + 1)]
                                    for mm in range(6):
                                        nc.tensor.matmul(
                                            gacc[mm][:],
                                            wihs[:, ki, mm, :],
                                            rhs,
                                            start=(ki == 0 and layer != 2),
                                            stop=(ki == nk - 1),
                                            skip_group_check=True,
                                        )
                                for mm in range(6):
                                    gslice = gi[
                                        :,
                                        slt * s : slt * (s + 1),
                                        mm,
                                        8 * half : 8 * (half + 1),
                                    ]
                                    nc.scalar.activation(
                                        gslice,
                                        gacc[mm][:].rearrange("p (a b) -> p a b", b=8),
                                        AF.Identity,
                                        bias=gbsb[:, layer, half, mm : mm + 1],
                                    )

                    if layer == 0:
                        # layer-2 G-part gi: fill scan-0's idle PE slots.
                        # Work closures are pumped a few per scan step.
                        l2g_ps = tc.tile_pool(name="l2g", bufs=1, space="PSUM")
                        pl2 = l2g_ps.__enter__()
                        l2acc = [
                            pl2.tile([128, SL], f32, tag=f"l2acc{i}", name=f"l2acc{i}")
                            for i in range(4)
                        ]
                        l2spill = tc.tile_pool(name="l2sp", bufs=3)
                        pl2s = l2spill.__enter__()

                        def mk_l2g():
                            nk2 = 16
                            for half in range(2):
                                wihs2 = pw.tile(
                                    [128, nk2, 6, 128], f32r, tag="wihs", name="wihs2"
                                )
                                dma = nc.sync.dma_start(
                                    out=wihs2[:],
                                    in_=wih_d[2][
                                        : 128 * nk2, 768 * half : 768 * (half + 1)
                                    ]
                                    .rearrange(
                                        "(ki p) (mm j) -> p ki mm j", p=128, j=128
                                    )
                                    .bitcast(f32r),
                                )
                                for gi0, gn in ((0, 4), (4, 2)):
                                    for s in range(nsl):
                                        for ki in range(nk2):
                                            rt = pg.tile(
                                                [128, SL], f32r, tag="gs", name="rt2"
                                            )
                                            nc.sync.dma_start(
                                                out=rt[:],
                                                in_=gt_d[
                                                    ki, :, SL * s : SL * (s + 1)
                                                ].bitcast(f32r),
                                            )
                                            for mi in range(gn):
                                                yield order("PE", nc.tensor.matmul(
                                                    l2acc[mi][:],
                                                    wihs2[:, ki, gi0 + mi, :],
                                                    rt[:],
                                                    start=(ki == 0),
                                                    stop=(ki == nk2 - 1),
                                                ))
                                        for mi in range(gn):
                                            stg = pl2s.tile(
                                                [128, SL], bf16, tag="stg", name="stg"
                                            )
                                            nc.vector.tensor_copy(
                                                stg[:], l2acc[mi][:]
                                            )
                                            nc.sync.dma_start(
                                                out=gi2_d[
                                                    :, half, gi0 + mi,
                                                    SL * s : SL * (s + 1),
                                                ],
                                                in_=stg[:],
                                            )
                                            yield None

                        l2g_gen = mk_l2g()

                        def l2g_pump(k):
                            for _ in range(k):
                                if l2g_gen is None:
                                    return
                                try:
                                    next(l2g_gen)
                                except StopIteration:
                                    return
                    else:
                        l2g_pump = None

                    # ---- scan: 2 staggered chains (fwd, bwd),
                    # gates-on-partition, eye-matmul psum preloads
                    with (
                        tc.tile_pool(name="psf", bufs=2, space="PSUM") as ppf,
                        tc.tile_pool(name="psb", bufs=2, space="PSUM") as ppb,
                    ):
                        hprev = [[h0f[:]], [h0b[:]]]
                        hmat = [h0f[:], h0b[:]]
                        Pcur = [None, None]

                        def emit_pre(d, step):
                            pp = ppf if d == 0 else ppb
                            t = step if d == 0 else t_len - 1 - step
                            cs = 8 * d
                            Pfull = pp.tile(
                                [128, 64, 8], f32, tag=f"P{d}", name=f"Pfull{d}"
                            )
                            P = Pfull[:, 0:6, :]
                            order("PE", nc.tensor.matmul(
                                P[:, 0:4, :], eyeb[:], gi[:, t, 0:4, cs : cs + 8],
                                start=True, stop=False, skip_group_check=True,
                            ))
                            order("PE", nc.tensor.matmul(
                                P[:, 4:6, :], eyeb[:], bhnt[:, layer, :, cs : cs + 8],
                                start=False, stop=False, skip_group_check=True,
                            ))
                            Pcur[d] = P

                        def emit(d, step):
                            t = step if d == 0 else t_len - 1 - step
                            cs = 8 * d
                            P = Pcur[d]
                            for m in range(6):
                                for kc in range(2):
                                    for hpart in hprev[d]:
                                        order("PE", nc.tensor.matmul(
                                            P[:, m, :],
                                            whhs[:, layer, d, kc, m, :],
                                            hpart[:, kc, :],
                                            start=False,
                                            stop=(kc == 1 and hpart is hprev[d][-1]),
                                            skip_group_check=True,
                                        ))
                            if step + 1 < t_len:
                                emit_pre(d, step + 1)
                            rz = pscr.tile(
                                [128, 4, 8], bf16, tag=f"rz{d}", name=f"rz{d}"
                            )
                            order("ACT", nc.scalar.activation(
                                rz[:], P[:, 0:4, :], AF.Sigmoid
                            ))
                            ntl = pscr.tile(
                                [128, 2, 8], f32, tag=f"ntl{d}", name=f"ntl{d}"
                            )
                            order("DVE", nc.vector.tensor_tensor(
                                ntl[:], P[:, 4:6, :], rz[:, 0:2, :], ALU.mult
                            ))
                            order("DVE", nc.vector.tensor_tensor(
                                ntl[:], ntl[:], gi[:, t, 4:6, cs : cs + 8], ALU.add
                            ))
                            nn = pscr.tile(
                                [128, 2, 8], bf16, tag=f"nn{d}", name=f"nn{d}"
                            )
                            order("ACT", nc.scalar.activation(nn[:], ntl[:], AF.Tanh))
                            # h' = (1-z)*n + z*h; next matmul streams zh and u
                            # as two moving operands so h' is off-path
                            zh = pscr.tile(
                                [128, 2, 8], bf16, tag=f"zh{d}", name=f"zh{d}"
                            )
                            order("Pool", nc.gpsimd.tensor_tensor(
                                zh[:], rz[:, 2:4, :], hmat[d], ALU.mult
                            ))
                            zc = pscr.tile(
                                [128, 2, 8], bf16, tag=f"zc{d}", name=f"zc{d}"
                            )
                            order("Pool", nc.gpsimd.tensor_scalar(
                                out=zc[:], in0=rz[:, 2:4, :], scalar1=-1.0,
                                scalar2=1.0, op0=ALU.mult, op1=ALU.add,
                            ))
                            u = pscr.tile(
                                [128, 2, 8], bf16, tag=f"u{d}", name=f"u{d}"
                            )
                            order("DVE", nc.vector.tensor_tensor(
                                u[:], zc[:], nn[:], ALU.mult
                            ))
                            hnew = yTd[d][:, :, BL * t : BL * (t + 1)]
                            order("Pool", nc.gpsimd.tensor_tensor(
                                hnew, zh[:], u[:], ALU.add
                            ))
                            hprev[d] = [zh[:], u[:]]
                            hmat[d] = hnew

                        emit_pre(0, 0)
                        emit_pre(1, 0)
                        for step in range(t_len):
                            emit(0, step)
                            if l2g_pump is not None:
                                l2g_pump(2)
                            emit(1, step)
                            if l2g_pump is not None:
                                l2g_pump(2)
                        if layer == 0:
                            l2g_pump(10**6)
                        if layer < 3:
                            # reset h0 for the next layer (cheap, off-path)
                            nc.vector.memset(h0f[:], 0.0)
                            nc.vector.memset(h0b[:], 0.0)
                    if layer == 0:
                        l2spill.__exit__(None, None, None)
                        l2g_ps.__exit__(None, None, None)

                # output: [hb, hf] per sequence; finals live in yT slices
                for hh, dd in ((0, 1), (1, 0)):
                    src_y = yTd[dd]
                    tslice = (
                        slice(0, BL) if dd == 1 else slice(BL * (t_len - 1), BL * t_len)
                    )
                    for chh in range(2):
                        c0 = 256 * hh + 128 * chh
                        ov = out_d[:, c0 : c0 + 128].rearrange("b p -> p b")
                        nc.sync.dma_start(
                            out=ov, in_=src_y[:, chh, tslice].bitcast(f32)
                        )

    return nc


# ---------------------------------------------------------------------------
# host-side weight prep
# ---------------------------------------------------------------------------
def _prep_weights(inputs):
    import ml_dtypes
    bf = ml_dtypes.bfloat16
    names = ["mod0", "mod1", "rep0", "rep1"]
    wih = []
    whhs = np.empty((128, 4, 2, 2, 6, 128), np.float32)
    gb = np.empty((128, 4, 2, 6), np.float32)
    bhnt = np.empty((128, 4, 2, 16), np.float32)
    for layer, nm in enumerate(names):
        Wih = np.asarray(inputs[f"{nm}_Wih"], np.float32)   # [2, 768, in]
        Whh = np.asarray(inputs[f"{nm}_Whh"], np.float32)   # [2, 768, 256]
        bb = np.asarray(inputs[f"{nm}_b"], np.float32)      # [2, 2, 768]
        wih.append(
            np.ascontiguousarray(np.concatenate([Wih[0].T, Wih[1].T], axis=1))
        )
        for d in range(2):
            # whhs[p, l, d, kc, m, j] = Whh[d].T[kc*128+p, m*128+j]
            WT = Whh[d].T.reshape(2, 128, 6, 128)
            whhs[:, layer, d] = WT.transpose(1, 0, 2, 3)
            vec = bb[d, 0] + np.concatenate([bb[d, 1][:D2], np.zeros(D, np.float32)])
            gb[:, layer, d, :] = vec.reshape(6, 128).T
            bhnt[:, layer, :, 8 * d : 8 * d + 8] = (
                bb[d, 1][D2:].reshape(2, 128).T[:, :, None]
            )
    return wih, whhs.astype(bf), gb, bhnt.astype(bf)


_PROG = None


def kernel(**inputs):
    global _PROG
    if _PROG is None:
        _PROG = build_program()
    nc = _PROG

    import ml_dtypes

    wih, whhs, gb, bhnt = _prep_weights(inputs)
    ws = np.asarray(inputs["Ws"], np.float32).reshape(3, D2)
    eye = np.eye(128, dtype=np.float32)
    c_all = np.asarray(inputs["embd_context"], np.float32)
    q_all = np.asarray(inputs["embd_query"], np.float32)

    shared = {
        "eye": eye,
        "eyeb": np.eye(128, dtype=ml_dtypes.bfloat16),
        "wsplit": np.ascontiguousarray(ws),
        "whhs": whhs,
        "gbias": gb,
        "bhnt": bhnt,
    }
    for layer in range(4):
        shared[f"wih{layer}"] = wih[layer]

    in_maps = []
    for i in range(NCORES):
        ci = c_all[BL * i : BL * (i + 1)]           # [8, 256, 512]
        c_tm = np.ascontiguousarray(
            ci.transpose(1, 0, 2).reshape(T * BL, D2)
        )
        qi = np.ascontiguousarray(
            q_all[BL * i : BL * (i + 1)].reshape(NQTOK, D2)
        )
        m = dict(shared)
        m["c"] = c_tm
        m["q"] = qi
        in_maps.append(m)

    res = run_bass_kernel_spmd(nc, in_maps, core_ids=list(range(NCORES)))
    out = np.concatenate([res.results[i]["out"] for i in range(NCORES)], axis=0)
    return np.ascontiguousarray(out.astype(np.float32))

